# revision 11
# baseline (speedup 1.0000x reference)
"""Distributed causal self-attention kernel for 8 TRN2 NeuronCores.

Problem (hardcoded): B=4, T=2048, C=1024, H=16 heads, D=64 head dim, fp32.
  y = softmax(causal(x Wq^T (x Wk^T)^T / sqrt(D))) (x Wv^T) Wp^T + biases

Sharding: data-parallel over B (4 groups) x tensor-parallel over heads
(2 groups of 8 heads).  Core c handles batch c//2, head-group c%2.  Each
core computes a partial output projection y_partial = O_g @ Wp[:,cols_g]^T;
the host sums the two partials of each batch pair (the 2-way all-reduce of
the sharding hint) and adds bp.

Per-core kernel (all matmuls in float32r -- full PE rate, ~1e-4 rel err):
  phase 1: PE-transpose x[b] and the weight slices, project
           Q^T[j,t], K^T[j,t] (j on partitions) and V[t,j] (t on partitions).
  phase 2: per head pair / 512-query tile: S^T[k,q] = K^T.T Q^T tiles
           (two heads co-computed via PE row tiling), exp on ScalarE
           (scale=1/8 fused), causal mask via gpsimd.affine_select on the
           4 diagonal k-tiles, O^T accumulation with a ones-column
           appended to V so PSUM row 64 is the softmax denominator,
           then O^T[d,q] * (1/denom) broadcast (tiny K=1 matmul).
  phase 3: y_partial[t, :] = O^T.T @ Wp_g^T  (contracts j=8*64 head dims).
"""

import numpy as np

import concourse.bass as bass
import concourse.mybir as mybir
from concourse.tile import TileContext
from concourse.bass_utils import run_bass_kernel_spmd

F32 = mybir.dt.float32
F32R = mybir.dt.float32r
AF = mybir.ActivationFunctionType
ALU = mybir.AluOpType

P = 128          # partitions
T = 2048         # sequence length
C = 1024         # model dim
D = 64           # head dim
HG = 8           # heads per core
J = HG * D       # per-core projection width (512)
CC = C // P      # contraction chunks over model dim (8)
JC = J // P      # j chunks (4)
NT = T // P      # 128-row t tiles (16)
TBS = 256        # t block size for x^T staging
NTB = T // TBS   # t blocks (8)
NQ = T // 512    # 512-wide query tiles (4)
NPAIR = HG // 2  # co-computed head pairs (4)

_CACHE = {}


def _split_excess_waits(nc):
    """Walrus in this container only accepts 1 sync-wait on CTRL-queue
    instructions (Drain etc.).  Hoist excess waits onto preceding nops on
    the same engine queue (program order makes this equivalent)."""
    n = 0
    for f in nc.m.functions:
        for bb in f.blocks:
            out = []
            for inst in bb.instructions:
                si = inst.sync_info
                limit = 1
                if si is not None and si.on_wait and len(si.on_wait) > limit:
                    waits = list(si.on_wait)
                    excess, keep = waits[:-limit], waits[-limit:]
                    for ci in range(0, len(excess), limit):
                        n += 1
                        out.append(mybir.InstNoOp(
                            name=f"waitsplit_{n}", opcode="nop", engine=inst.engine,
                            sync_info=mybir.SyncInfo(
                                on_wait=excess[ci:ci + limit], on_update=[]),
                        ))
                    inst.sync_info = mybir.SyncInfo(
                        on_wait=keep, on_update=list(si.on_update))
                out.append(inst)
            bb.instructions = out


def _r(ap):
    return ap


def _build(debug=False):
    nc = bass.Bass()
    x_in = nc.dram_tensor("x", [T, C], F32, kind="ExternalInput")
    wq_in = nc.dram_tensor("wq", [J, C], F32, kind="ExternalInput")
    wk_in = nc.dram_tensor("wk", [J, C], F32, kind="ExternalInput")
    wv_in = nc.dram_tensor("wv", [J, C], F32, kind="ExternalInput")
    wp_in = nc.dram_tensor("wp", [C, J], F32, kind="ExternalInput")
    bq_in = nc.dram_tensor("bq", [J], F32, kind="ExternalInput")
    bk_in = nc.dram_tensor("bk", [J], F32, kind="ExternalInput")
    bv_in = nc.dram_tensor("bv", [J], F32, kind="ExternalInput")
    y_out = nc.dram_tensor("y", [T, C], F32, kind="ExternalOutput")
    if debug:
        qt_d = nc.dram_tensor("qt_d", [P, JC, T], F32, kind="ExternalOutput")
        kt_d = nc.dram_tensor("kt_d", [P, JC, T], F32, kind="ExternalOutput")
        v_d = nc.dram_tensor("v_d", [P, NT, HG, D + 1], F32, kind="ExternalOutput")
        ot_d = nc.dram_tensor("ot_d", [P, JC, T], F32, kind="ExternalOutput")

    with TileContext(nc) as tc:
        with tc.tile_pool(name="persist", bufs=1) as persist:
            # persistent tensors (per-partition: 32+32+33+~3 KB)
            qt_t = persist.tile([P, JC, T], F32R, tag="qt")   # Q^T
            kt_t = persist.tile([P, JC, T], F32R, tag="kt")   # K^T
            v_t = persist.tile([P, NT, HG, D + 1], F32R, tag="v")  # V + ones col
            ident = persist.tile([P, P], F32, tag="ident")
            ones_row = persist.tile([1, P], F32R, tag="ones")
            bq_sb = persist.tile([P, JC], F32, tag="bq")
            bk_sb = persist.tile([P, JC], F32, tag="bk")
            bv_sb = persist.tile([1, J], F32, tag="bv")
            bv_r = persist.tile([1, J], F32R, tag="bvr")
            bv_bc = persist.tile([P, J], F32, tag="bvbc")

            # constants
            from concourse.masks import make_identity
            make_identity(nc, ident[:])
            ones_f32 = persist.tile([P, P], F32, tag="ones_f32")
            nc.gpsimd.memset(ones_f32[:], 1.0)
            nc.vector.tensor_copy(ones_row[:], ones_f32[0:1, :])
            nc.vector.tensor_copy(
                v_t[:, :, :, D:D + 1],
                ones_f32[:, None, None, 0:1].to_broadcast((P, NT, HG, 1)))
            nc.sync.dma_start(bq_sb[:], bq_in.rearrange("(o p) -> p o", p=P))
            nc.sync.dma_start(bk_sb[:], bk_in.rearrange("(o p) -> p o", p=P))
            nc.sync.dma_start(bv_sb[:], bv_in[None, :])

            # ---------------- phase 1: transposes + QKV projections ---------
            with (
                tc.tile_pool(name="nat", bufs=3) as nat_pool,
                tc.tile_pool(name="xt", bufs=2) as xt_pool,
                tc.tile_pool(name="wt", bufs=1) as wt_pool,
                tc.tile_pool(name="ps_tr", bufs=3, space="PSUM") as ps_tr,
                tc.tile_pool(name="ps_mm", bufs=3, space="PSUM") as ps_mm,
            ):
                # bv broadcast to all 128 partitions via K=1 matmul
                nc.vector.tensor_copy(bv_r[:], bv_sb[:])
                ps_bv = ps_mm.tile([P, J], F32, tag="mm")
                nc.tensor.matmul(ps_bv[:], lhsT=ones_row[:], rhs=bv_r[:],
                                 start=True, stop=True)
                nc.vector.tensor_copy(bv_bc[:], ps_bv[:])

                # weight transposes: w[j, c] -> w^T[c, j] chunks [P, CC, J]
                wts = {}
                for name, w_in in (("q", wq_in), ("k", wk_in), ("v", wv_in)):
                    wt = wt_pool.tile([P, CC, J], F32R, tag=f"w{name}t")
                    wts[name] = wt
                    for jt in range(JC):
                        wnat = nat_pool.tile([P, C], F32, tag="nat")
                        nc.sync.dma_start(wnat[:], w_in[jt * P:(jt + 1) * P, :])
                        for cc in range(CC):
                            pst = ps_tr.tile([P, P], F32, tag="tr")
                            nc.tensor.transpose(
                                pst[:], _r(wnat[:, cc * P:(cc + 1) * P]), _r(ident[:]))
                            nc.vector.tensor_copy(
                                wt[:, cc, jt * P:(jt + 1) * P], pst[:])

                # x^T per 256-column t block; project Q^T, K^T, V
                for tb in range(NTB):
                    xt = xt_pool.tile([P, CC, TBS], F32R, tag="xt")
                    for sub in range(TBS // P):
                        tt = tb * (TBS // P) + sub
                        xnat = nat_pool.tile([P, C], F32, tag="nat")
                        nc.sync.dma_start(xnat[:], x_in[tt * P:(tt + 1) * P, :])
                        for cc in range(CC):
                            pst = ps_tr.tile([P, P], F32, tag="tr")
                            nc.tensor.transpose(
                                pst[:], _r(xnat[:, cc * P:(cc + 1) * P]), _r(ident[:]))
                            nc.vector.tensor_copy(
                                xt[:, cc, sub * P:(sub + 1) * P], pst[:])

                    # Q^T[j, t] and K^T[j, t]
                    for name, dest, bias in (("q", qt_t, bq_sb), ("k", kt_t, bk_sb)):
                        wt = wts[name]
                        for jc in range(JC):
                            psq_full = ps_mm.tile([P, J], F32, tag="mm",
                                                  name=f"psq_{tb}_{name}_{jc}")
                            psq = psq_full[:, :TBS]
                            for cc in range(CC):
                                nc.tensor.matmul(
                                    psq[:],
                                    lhsT=_r(wt[:, cc, jc * P:(jc + 1) * P]),
                                    rhs=_r(xt[:, cc, :]),
                                    start=(cc == 0), stop=(cc == CC - 1))
                            nc.scalar.activation(
                                dest[:, jc, tb * TBS:(tb + 1) * TBS], psq[:],
                                AF.Identity, bias=bias[:, jc:jc + 1])

                    # V[t, j] (+ per-j bias broadcast over t)
                    for sub in range(TBS // P):
                        tt = tb * (TBS // P) + sub
                        psv = ps_mm.tile([P, J], F32, tag="mm")
                        for cc in range(CC):
                            nc.tensor.matmul(
                                psv[:],
                                lhsT=_r(xt[:, cc, sub * P:(sub + 1) * P]),
                                rhs=_r(wts["v"][:, cc, :]),
                                start=(cc == 0), stop=(cc == CC - 1))
                        nc.vector.tensor_tensor(
                            v_t[:, tt, :, 0:D],
                            psv.rearrange("p (h d) -> p h d", h=HG),
                            bv_bc.rearrange("p (h d) -> p h d", h=HG),
                            ALU.add)

            # ---------------- phases 2+3 -----------------------------------
            with tc.tile_pool(name="persist2", bufs=1) as persist2:
                ot_t = persist2.tile([P, JC, T], F32, tag="ot")  # O^T
                ot_r = persist2.tile([P, JC, T], F32R, tag="otr")

                with (
                    tc.tile_pool(name="e", bufs=4) as e_pool,
                    tc.tile_pool(name="tmp", bufs=2) as tmp_pool,
                    tc.tile_pool(name="rc", bufs=2) as rc_pool,
                    tc.tile_pool(name="ps_s", bufs=3, space="PSUM") as ps_s,
                    tc.tile_pool(name="ps_o", bufs=2, space="PSUM") as ps_o,
                    tc.tile_pool(name="ps_bc", bufs=2, space="PSUM") as ps_bc,
                ):
                    for pair in range(NPAIR):
                        for qt in range(NQ):
                            nk = (qt + 1) * 4
                            qs = slice(qt * 512, (qt + 1) * 512)
                            pso = [ps_o.tile([P, 512], F32, tag="o",
                                                 name=f"pso_{pair}_{qt}_{i}")
                                   for i in range(2)]
                            for kc in range(nk):
                                ks = slice(kc * P, (kc + 1) * P)
                                es = []
                                for half in range(2):
                                    hp = slice(half * 64, half * 64 + 64)
                                    pss = ps_s.tile([P, 512], F32, tag="s")
                                    nc.tensor.matmul(
                                        pss[:],
                                        lhsT=_r(kt_t[hp, pair, ks]),
                                        rhs=_r(qt_t[hp, pair, qs]),
                                        start=True, stop=True)
                                    e = e_pool.tile([P, 512], F32R, tag="e")
                                    nc.scalar.activation(
                                        e[:], pss[:], AF.Exp, scale=0.125)
                                    if kc >= qt * 4:  # diagonal: causal mask
                                        delta = (kc - qt * 4) * P
                                        nc.gpsimd.affine_select(
                                            out=e[:], in_=e[:],
                                            compare_op=ALU.is_ge, fill=0.0,
                                            base=-delta, channel_multiplier=-1,
                                            pattern=[[1, 512]])
                                    es.append(e)
                                for half in range(2):
                                    h = pair * 2 + half
                                    nc.tensor.matmul(
                                        pso[half][0:D + 1, :],
                                        lhsT=_r(v_t[:, kc, h, :]),
                                        rhs=_r(es[half][:]),
                                        start=(kc == 0), stop=(kc == nk - 1))
                            for half in range(2):
                                recip = rc_pool.tile([1, 512], F32R, tag="rc")
                                with nc.allow_low_precision(
                                        reason="f32r recip, ~1e-4 rel err ok"):
                                    nc.vector.reciprocal(
                                        recip[:], pso[half][D:D + 1, :])
                                psb = ps_bc.tile([D, 512], F32, tag="bc")
                                nc.tensor.matmul(
                                    psb[:], lhsT=_r(ones_row[0:1, 0:D]),
                                    rhs=_r(recip[:]), start=True, stop=True)
                                bc_sb = rc_pool.tile([D, 512], F32, tag="bcsb")
                                nc.scalar.copy(bc_sb[:], psb[:])
                                if half == 0:
                                    nc.vector.tensor_tensor(
                                        ot_t[0:D, pair, qs],
                                        pso[half][0:D, :], bc_sb[:], ALU.mult)
                                else:
                                    tmp = tmp_pool.tile([D, 512], F32, tag="tmp")
                                    nc.vector.tensor_tensor(
                                        tmp[:], pso[half][0:D, :], bc_sb[:], ALU.mult)
                                    nc.sync.dma_start(
                                        ot_t[D:2 * D, pair, qs], tmp[:])

                if debug:
                    nc.sync.dma_start(qt_d[:], qt_t[:].bitcast(F32))
                    nc.sync.dma_start(kt_d[:], kt_t[:].bitcast(F32))
                    nc.sync.dma_start(v_d[:], v_t[:].bitcast(F32))
                    nc.sync.dma_start(ot_d[:], ot_t[:])
                # ------------ phase 3: output projection --------------------
                with (
                    tc.tile_pool(name="nat3", bufs=2) as nat3,
                    tc.tile_pool(name="wpt", bufs=1) as wpt_pool,
                    tc.tile_pool(name="yout", bufs=2) as y_pool,
                    tc.tile_pool(name="ps_tr3", bufs=2, space="PSUM") as ps_tr3,
                    tc.tile_pool(name="ps_y", bufs=4, space="PSUM") as ps_y,
                ):
                    nc.vector.tensor_copy(ot_r[:], ot_t[:])
                    wpt = wpt_pool.tile([P, JC, C], F32R, tag="wpt")
                    for ct in range(C // P):
                        wnat = nat3.tile([P, J], F32, tag="nat3")
                        nc.sync.dma_start(wnat[:], wp_in[ct * P:(ct + 1) * P, :])
                        for jc in range(JC):
                            pst = ps_tr3.tile([P, P], F32, tag="tr3")
                            nc.tensor.transpose(
                                pst[:], _r(wnat[:, jc * P:(jc + 1) * P]), _r(ident[:]))
                            nc.vector.tensor_copy(
                                wpt[:, jc, ct * P:(ct + 1) * P], pst[:])

                    for tt in range(NT):
                        ts = slice(tt * P, (tt + 1) * P)
                        ytile = y_pool.tile([P, C], F32, tag="y")
                        for nh in range(2):
                            psy = ps_y.tile([P, 512], F32, tag="ps_y")
                            for jc in range(JC):
                                nc.tensor.matmul(
                                    psy[:],
                                    lhsT=ot_r[:, jc, ts],
                                    rhs=_r(wpt[:, jc, nh * 512:(nh + 1) * 512]),
                                    start=(jc == 0), stop=(jc == JC - 1))
                            nc.vector.tensor_copy(
                                ytile[:, nh * 512:(nh + 1) * 512], psy[:])
                        nc.sync.dma_start(y_out[ts, :], ytile[:])

    _split_excess_waits(nc)
    return nc


def _get_nc():
    if "nc" not in _CACHE:
        _CACHE["nc"] = _build()
    return _CACHE["nc"]


def kernel(x, Wq, bq, Wk, bk, Wv, bv, Wp, bp, **_unused):
    x = np.ascontiguousarray(np.asarray(x, dtype=np.float32))
    Wq = np.asarray(Wq, dtype=np.float32)
    Wk = np.asarray(Wk, dtype=np.float32)
    Wv = np.asarray(Wv, dtype=np.float32)
    Wp = np.asarray(Wp, dtype=np.float32)
    bq = np.asarray(bq, dtype=np.float32)
    bk = np.asarray(bk, dtype=np.float32)
    bv = np.asarray(bv, dtype=np.float32)
    bp = np.asarray(bp, dtype=np.float32)

    nc = _get_nc()
    in_maps = []
    for c in range(8):
        b, g = c // 2, c % 2
        js = slice(g * J, (g + 1) * J)
        in_maps.append({
            "x": np.ascontiguousarray(x[b]),
            "wq": np.ascontiguousarray(Wq[js, :]),
            "wk": np.ascontiguousarray(Wk[js, :]),
            "wv": np.ascontiguousarray(Wv[js, :]),
            "wp": np.ascontiguousarray(Wp[:, js]),
            "bq": np.ascontiguousarray(bq[js]),
            "bk": np.ascontiguousarray(bk[js]),
            "bv": np.ascontiguousarray(bv[js]),
        })
    res = run_bass_kernel_spmd(nc, in_maps, list(range(8)))
    out = np.empty((4, T, C), dtype=np.float32)
    for b in range(4):
        out[b] = res.results[2 * b]["y"] + res.results[2 * b + 1]["y"] + bp
    return out


# revision 18
# speedup vs baseline: 1.0321x; 1.0321x over previous
"""Distributed causal self-attention kernel for 8 TRN2 NeuronCores.

Problem (hardcoded): B=4, T=2048, C=1024, H=16 heads, D=64 head dim, fp32.
  y = softmax(causal(x Wq^T (x Wk^T)^T / sqrt(D))) (x Wv^T) Wp^T + biases

Sharding: data-parallel over B (4 groups) x tensor-parallel over heads
(2 groups of 8 heads).  Core c handles batch c//2, head-group c%2.  Each
core computes a partial output projection y_partial = O_g @ Wp[:,cols_g]^T;
the host sums the two partials of each batch pair (the 2-way all-reduce of
the sharding hint) and adds bp.

Per-core kernel (all matmuls in float32r -- full PE rate, ~1e-4 rel err):
  phase 1: PE-transpose x[b] and the weight slices, project
           Q^T[j,t], K^T[j,t] (j on partitions) and V[t,j] (t on partitions).
  phase 2: per head pair / 512-query tile: S^T[k,q] = K^T.T Q^T tiles
           (two heads co-computed via PE row tiling), exp on ScalarE
           (scale=1/8 fused), causal mask via gpsimd.affine_select on the
           4 diagonal k-tiles, O^T accumulation with a ones-column
           appended to V so PSUM row 64 is the softmax denominator,
           then O^T[d,q] * (1/denom) broadcast (tiny K=1 matmul).
  phase 3: y_partial[t, :] = O^T.T @ Wp_g^T  (contracts j=8*64 head dims).
"""

import numpy as np

import concourse.bass as bass
import concourse.mybir as mybir
from concourse.tile import TileContext
from concourse.bass_utils import run_bass_kernel_spmd

F32 = mybir.dt.float32
F32R = mybir.dt.float32r
AF = mybir.ActivationFunctionType
ALU = mybir.AluOpType

P = 128          # partitions
T = 2048         # sequence length
C = 1024         # model dim
D = 64           # head dim
HG = 8           # heads per core
J = HG * D       # per-core projection width (512)
CC = C // P      # contraction chunks over model dim (8)
JC = J // P      # j chunks (4)
NT = T // P      # 128-row t tiles (16)
TBS = 256        # t block size for x^T staging
NTB = T // TBS   # t blocks (8)
NQ = T // 512    # 512-wide query tiles (4)
NPAIR = HG // 2  # co-computed head pairs (4)

_CACHE = {}


def _split_excess_waits(nc):
    """Walrus in this container only accepts 1 sync-wait on CTRL-queue
    instructions (Drain etc.).  Hoist excess waits onto preceding nops on
    the same engine queue (program order makes this equivalent)."""
    n = 0
    for f in nc.m.functions:
        for bb in f.blocks:
            out = []
            for inst in bb.instructions:
                si = inst.sync_info
                limit = 1
                if si is not None and si.on_wait and len(si.on_wait) > limit:
                    waits = list(si.on_wait)
                    excess, keep = waits[:-limit], waits[-limit:]
                    for ci in range(0, len(excess), limit):
                        n += 1
                        out.append(mybir.InstNoOp(
                            name=f"waitsplit_{n}", opcode="nop", engine=inst.engine,
                            sync_info=mybir.SyncInfo(
                                on_wait=excess[ci:ci + limit], on_update=[]),
                        ))
                    inst.sync_info = mybir.SyncInfo(
                        on_wait=keep, on_update=list(si.on_update))
                out.append(inst)
            bb.instructions = out


def _r(ap):
    return ap


def _build(debug=False):
    nc = bass.Bass()
    x_in = nc.dram_tensor("x", [T, C], F32, kind="ExternalInput")
    wq_in = nc.dram_tensor("wq", [J, C], F32, kind="ExternalInput")
    wk_in = nc.dram_tensor("wk", [J, C], F32, kind="ExternalInput")
    wv_in = nc.dram_tensor("wv", [J, C], F32, kind="ExternalInput")
    wp_in = nc.dram_tensor("wp", [C, J], F32, kind="ExternalInput")
    bq_in = nc.dram_tensor("bq", [J], F32, kind="ExternalInput")
    bk_in = nc.dram_tensor("bk", [J], F32, kind="ExternalInput")
    bv_in = nc.dram_tensor("bv", [J], F32, kind="ExternalInput")
    y_out = nc.dram_tensor("y", [T, C], F32, kind="ExternalOutput")
    if debug:
        qt_d = nc.dram_tensor("qt_d", [P, JC, T], F32, kind="ExternalOutput")
        kt_d = nc.dram_tensor("kt_d", [P, JC, T], F32, kind="ExternalOutput")
        v_d = nc.dram_tensor("v_d", [P, NT, HG, D + 1], F32, kind="ExternalOutput")
        ot_d = nc.dram_tensor("ot_d", [P, JC, T], F32, kind="ExternalOutput")

    with TileContext(nc) as tc:
        with tc.tile_pool(name="persist", bufs=1) as persist:
            # persistent tensors (per-partition: 32+32+33+~3 KB)
            qt_t = persist.tile([P, JC, T], F32R, tag="qt")   # Q^T
            kt_t = persist.tile([P, JC, T], F32R, tag="kt")   # K^T
            v_t = persist.tile([P, NT, HG, D + 1], F32R, tag="v")  # V + ones col
            ident = persist.tile([P, P], F32, tag="ident")
            ones_row = persist.tile([1, P], F32R, tag="ones")
            bq_sb = persist.tile([P, JC], F32, tag="bq")
            bk_sb = persist.tile([P, JC], F32, tag="bk")
            bv_sb = persist.tile([1, J], F32, tag="bv")
            bv_r = persist.tile([1, J], F32R, tag="bvr")
            bv_bc = persist.tile([P, J], F32, tag="bvbc")

            # constants
            from concourse.masks import make_identity
            make_identity(nc, ident[:])
            ones_f32 = persist.tile([P, P], F32, tag="ones_f32")
            nc.gpsimd.memset(ones_f32[:], 1.0)
            nc.vector.tensor_copy(ones_row[:], ones_f32[0:1, :])
            nc.vector.tensor_copy(
                v_t[:, :, :, D:D + 1],
                ones_f32[:, None, None, 0:1].to_broadcast((P, NT, HG, 1)))
            nc.sync.dma_start(bq_sb[:], bq_in.rearrange("(o p) -> p o", p=P))
            nc.sync.dma_start(bk_sb[:], bk_in.rearrange("(o p) -> p o", p=P))
            nc.sync.dma_start(bv_sb[:], bv_in[None, :])

            # ---------------- phase 1: transposes + QKV projections ---------
            with (
                tc.tile_pool(name="nat", bufs=3) as nat_pool,
                tc.tile_pool(name="xt", bufs=2) as xt_pool,
                tc.tile_pool(name="wt", bufs=1) as wt_pool,
                tc.tile_pool(name="ps_tr", bufs=3, space="PSUM") as ps_tr,
                tc.tile_pool(name="ps_mm", bufs=3, space="PSUM") as ps_mm,
            ):
                # bv broadcast to all 128 partitions via K=1 matmul
                nc.vector.tensor_copy(bv_r[:], bv_sb[:])
                ps_bv = ps_mm.tile([P, J], F32, tag="mm")
                nc.tensor.matmul(ps_bv[:], lhsT=ones_row[:], rhs=bv_r[:],
                                 start=True, stop=True)
                nc.vector.tensor_copy(bv_bc[:], ps_bv[:])

                # weight transposes: w[j, c] -> w^T[c, j] chunks [P, CC, J]
                wts = {}
                for name, w_in in (("q", wq_in), ("k", wk_in), ("v", wv_in)):
                    wt = wt_pool.tile([P, CC, J], F32R, tag=f"w{name}t")
                    wts[name] = wt
                    for jt in range(JC):
                        wnat = nat_pool.tile([P, C], F32, tag="nat")
                        nc.sync.dma_start(wnat[:], w_in[jt * P:(jt + 1) * P, :])
                        for cc in range(CC):
                            pst = ps_tr.tile([P, P], F32, tag="tr")
                            nc.tensor.transpose(
                                pst[:], _r(wnat[:, cc * P:(cc + 1) * P]), _r(ident[:]))
                            nc.vector.tensor_copy(
                                wt[:, cc, jt * P:(jt + 1) * P], pst[:])

                # x^T per 256-column t block; project Q^T, K^T, V
                for tb in range(NTB):
                    xt = xt_pool.tile([P, CC, TBS], F32R, tag="xt")
                    for sub in range(TBS // P):
                        tt = tb * (TBS // P) + sub
                        xnat = nat_pool.tile([P, C], F32, tag="nat")
                        nc.sync.dma_start(xnat[:], x_in[tt * P:(tt + 1) * P, :])
                        for cc in range(CC):
                            pst = ps_tr.tile([P, P], F32, tag="tr")
                            nc.tensor.transpose(
                                pst[:], _r(xnat[:, cc * P:(cc + 1) * P]), _r(ident[:]))
                            nc.vector.tensor_copy(
                                xt[:, cc, sub * P:(sub + 1) * P], pst[:])

                    # Q^T[j, t] and K^T[j, t]
                    for name, dest, bias in (("q", qt_t, bq_sb), ("k", kt_t, bk_sb)):
                        wt = wts[name]
                        for jc in range(JC):
                            psq_full = ps_mm.tile([P, J], F32, tag="mm",
                                                  name=f"psq_{tb}_{name}_{jc}")
                            psq = psq_full[:, :TBS]
                            for cc in range(CC):
                                nc.tensor.matmul(
                                    psq[:],
                                    lhsT=_r(wt[:, cc, jc * P:(jc + 1) * P]),
                                    rhs=_r(xt[:, cc, :]),
                                    start=(cc == 0), stop=(cc == CC - 1))
                            nc.scalar.activation(
                                dest[:, jc, tb * TBS:(tb + 1) * TBS], psq[:],
                                AF.Identity, bias=bias[:, jc:jc + 1])

                    # V[t, j] (+ per-j bias broadcast over t)
                    for sub in range(TBS // P):
                        tt = tb * (TBS // P) + sub
                        psv = ps_mm.tile([P, J], F32, tag="mm")
                        for cc in range(CC):
                            nc.tensor.matmul(
                                psv[:],
                                lhsT=_r(xt[:, cc, sub * P:(sub + 1) * P]),
                                rhs=_r(wts["v"][:, cc, :]),
                                start=(cc == 0), stop=(cc == CC - 1))
                        nc.vector.tensor_tensor(
                            v_t[:, tt, :, 0:D],
                            psv.rearrange("p (h d) -> p h d", h=HG),
                            bv_bc.rearrange("p (h d) -> p h d", h=HG),
                            ALU.add)

            # ---------------- phases 2+3 -----------------------------------
            with tc.tile_pool(name="persist2", bufs=1) as persist2:
                ot_t = persist2.tile([P, JC, T], F32, tag="ot")  # O^T
                ot_r = persist2.tile([P, JC, T], F32R, tag="otr")

                with (
                    tc.tile_pool(name="e", bufs=6) as e_pool,
                    tc.tile_pool(name="tmp", bufs=3) as tmp_pool,
                    tc.tile_pool(name="rc", bufs=2) as rc_pool,
                    tc.tile_pool(name="ps_s", bufs=3, space="PSUM") as ps_s,
                    tc.tile_pool(name="ps_o", bufs=3, space="PSUM") as ps_o,
                    tc.tile_pool(name="ps_bc", bufs=2, space="PSUM") as ps_bc,
                ):
                    for pair in range(NPAIR):
                        for qt in range(NQ):
                            nk = (qt + 1) * 4
                            qs = slice(qt * 512, (qt + 1) * 512)
                            pso = [ps_o.tile([P, 512], F32, tag="o",
                                                 name=f"pso_{pair}_{qt}_{i}")
                                   for i in range(2)]
                            for kc in range(nk):
                                ks = slice(kc * P, (kc + 1) * P)
                                es = []
                                for half in range(2):
                                    hp = slice(half * 64, half * 64 + 64)
                                    pss = ps_s.tile([P, 512], F32, tag="s")
                                    nc.tensor.matmul(
                                        pss[:],
                                        lhsT=_r(kt_t[hp, pair, ks]),
                                        rhs=_r(qt_t[hp, pair, qs]),
                                        start=True, stop=True)
                                    e = e_pool.tile([P, 512], F32R, tag="e")
                                    nc.scalar.activation(
                                        e[:], pss[:], AF.Exp, scale=0.125)
                                    if kc >= qt * 4:  # diagonal: causal mask
                                        delta = (kc - qt * 4) * P
                                        nc.gpsimd.affine_select(
                                            out=e[:], in_=e[:],
                                            compare_op=ALU.is_ge, fill=0.0,
                                            base=-delta, channel_multiplier=-1,
                                            pattern=[[1, 512]])
                                    es.append(e)
                                for half in range(2):
                                    h = pair * 2 + half
                                    nc.tensor.matmul(
                                        pso[half][0:D + 1, :],
                                        lhsT=_r(v_t[:, kc, h, :]),
                                        rhs=_r(es[half][:]),
                                        start=(kc == 0), stop=(kc == nk - 1))
                            # evict unnormalized O^T; softmax denominator
                            # handled as 1/d = exp(-ln d): ln of the PSUM ones
                            # row, K=1 matmul broadcast over 64 partitions,
                            # exp(-x) on eviction, one full-width multiply.
                            for half in range(2):
                                hs = slice(half * 64, (half + 1) * 64)
                                if half == 0:
                                    nc.scalar.copy(
                                        ot_t[0:D, pair, qs], pso[half][0:D, :])
                                else:
                                    tmp = tmp_pool.tile([D, 512], F32, tag="tmp")
                                    nc.scalar.copy(tmp[:], pso[half][0:D, :])
                                    nc.sync.dma_start(
                                        ot_t[D:2 * D, pair, qs], tmp[:])
                                lnden = rc_pool.tile(
                                    [1, 512], F32R, tag="ln",
                                    name=f"ln_{pair}_{qt}_{half}")
                                nc.scalar.activation(
                                    lnden[:], pso[half][D:D + 1, :], AF.Ln)
                                psb = ps_bc.tile([P, 512], F32, tag="bc",
                                                 name=f"psb_{pair}_{qt}_{half}")
                                nc.tensor.matmul(
                                    psb[:, :], lhsT=ones_row[0:1, :],
                                    rhs=lnden[:], start=True, stop=True)
                                bcx = tmp_pool.tile([P, 512], F32, tag="bcx",
                                                    name=f"bcx_{pair}_{qt}_{half}")
                                nc.scalar.activation(
                                    bcx[hs, :], psb[hs, :], AF.Exp, scale=-1.0)
                                nc.vector.tensor_tensor(
                                    ot_r[hs, pair, qs], ot_t[hs, pair, qs],
                                    bcx[hs, :], ALU.mult)

                if debug:
                    nc.sync.dma_start(qt_d[:], qt_t[:].bitcast(F32))
                    nc.sync.dma_start(kt_d[:], kt_t[:].bitcast(F32))
                    nc.sync.dma_start(v_d[:], v_t[:].bitcast(F32))
                    nc.sync.dma_start(ot_d[:], ot_r[:].bitcast(F32))
                # ------------ phase 3: output projection --------------------
                with (
                    tc.tile_pool(name="nat3", bufs=2) as nat3,
                    tc.tile_pool(name="wpt", bufs=1) as wpt_pool,
                    tc.tile_pool(name="yout", bufs=2) as y_pool,
                    tc.tile_pool(name="ps_tr3", bufs=2, space="PSUM") as ps_tr3,
                    tc.tile_pool(name="ps_y", bufs=4, space="PSUM") as ps_y,
                ):
                    wpt = wpt_pool.tile([P, JC, C], F32R, tag="wpt")
                    for ct in range(C // P):
                        wnat = nat3.tile([P, J], F32, tag="nat3")
                        nc.sync.dma_start(wnat[:], wp_in[ct * P:(ct + 1) * P, :])
                        for jc in range(JC):
                            pst = ps_tr3.tile([P, P], F32, tag="tr3")
                            nc.tensor.transpose(
                                pst[:], _r(wnat[:, jc * P:(jc + 1) * P]), _r(ident[:]))
                            nc.vector.tensor_copy(
                                wpt[:, jc, ct * P:(ct + 1) * P], pst[:])

                    for tt in range(NT):
                        ts = slice(tt * P, (tt + 1) * P)
                        ytile = y_pool.tile([P, C], F32, tag="y")
                        for nh in range(2):
                            psy = ps_y.tile([P, 512], F32, tag="ps_y")
                            for jc in range(JC):
                                nc.tensor.matmul(
                                    psy[:],
                                    lhsT=ot_r[:, jc, ts],
                                    rhs=_r(wpt[:, jc, nh * 512:(nh + 1) * 512]),
                                    start=(jc == 0), stop=(jc == JC - 1))
                            nc.vector.tensor_copy(
                                ytile[:, nh * 512:(nh + 1) * 512], psy[:])
                        nc.sync.dma_start(y_out[ts, :], ytile[:])

    _split_excess_waits(nc)
    return nc


def _get_nc():
    if "nc" not in _CACHE:
        _CACHE["nc"] = _build()
    return _CACHE["nc"]


def kernel(x, Wq, bq, Wk, bk, Wv, bv, Wp, bp, **_unused):
    x = np.ascontiguousarray(np.asarray(x, dtype=np.float32))
    Wq = np.asarray(Wq, dtype=np.float32)
    Wk = np.asarray(Wk, dtype=np.float32)
    Wv = np.asarray(Wv, dtype=np.float32)
    Wp = np.asarray(Wp, dtype=np.float32)
    bq = np.asarray(bq, dtype=np.float32)
    bk = np.asarray(bk, dtype=np.float32)
    bv = np.asarray(bv, dtype=np.float32)
    bp = np.asarray(bp, dtype=np.float32)

    nc = _get_nc()
    in_maps = []
    for c in range(8):
        b, g = c // 2, c % 2
        js = slice(g * J, (g + 1) * J)
        in_maps.append({
            "x": np.ascontiguousarray(x[b]),
            "wq": np.ascontiguousarray(Wq[js, :]),
            "wk": np.ascontiguousarray(Wk[js, :]),
            "wv": np.ascontiguousarray(Wv[js, :]),
            "wp": np.ascontiguousarray(Wp[:, js]),
            "bq": np.ascontiguousarray(bq[js]),
            "bk": np.ascontiguousarray(bk[js]),
            "bv": np.ascontiguousarray(bv[js]),
        })
    res = run_bass_kernel_spmd(nc, in_maps, list(range(8)))
    out = np.empty((4, T, C), dtype=np.float32)
    for b in range(4):
        out[b] = res.results[2 * b]["y"] + res.results[2 * b + 1]["y"] + bp
    return out


# revision 20
# speedup vs baseline: 1.5408x; 1.4929x over previous
"""Distributed causal self-attention kernel for 8 TRN2 NeuronCores.

Problem (hardcoded): B=4, T=2048, C=1024, H=16 heads, D=64 head dim, fp32.
  y = softmax(causal(x Wq^T (x Wk^T)^T / sqrt(D))) (x Wv^T) Wp^T + biases

Sharding: data-parallel over B (4 groups) x tensor-parallel over heads
(2 groups of 8 heads).  Core c handles batch c//2, head-group c%2.  Each
core computes a partial output projection y_partial = O_g @ Wp[:,cols_g]^T;
the host sums the two partials of each batch pair (the 2-way all-reduce of
the sharding hint) and adds bp.

Per-core kernel (all matmuls in float32r -- full PE rate, ~1e-4 rel err):
  phase 1: PE-transpose x[b] and the weight slices, project
           Q^T[j,t], K^T[j,t] (j on partitions) and V[t,j] (t on partitions).
  phase 2: per head pair / 512-query tile: S^T[k,q] = K^T.T Q^T tiles
           (two heads co-computed via PE row tiling), exp on ScalarE
           (scale=1/8 fused), causal mask via gpsimd.affine_select on the
           4 diagonal k-tiles, O^T accumulation with a ones-column
           appended to V so PSUM row 64 is the softmax denominator,
           then O^T[d,q] * (1/denom) broadcast (tiny K=1 matmul).
  phase 3: y_partial[t, :] = O^T.T @ Wp_g^T  (contracts j=8*64 head dims).
"""

import numpy as np

import concourse.bass as bass
import concourse.mybir as mybir
from concourse.tile import TileContext
from concourse.bass_utils import run_bass_kernel_spmd

F32 = mybir.dt.float32
F32R = mybir.dt.float32r
AF = mybir.ActivationFunctionType
ALU = mybir.AluOpType

P = 128          # partitions
T = 2048         # sequence length
C = 1024         # model dim
D = 64           # head dim
HG = 8           # heads per core
J = HG * D       # per-core projection width (512)
CC = C // P      # contraction chunks over model dim (8)
JC = J // P      # j chunks (4)
NT = T // P      # 128-row t tiles (16)
TBS = 256        # t block size for x^T staging
NTB = T // TBS   # t blocks (8)
NQ = T // 512    # 512-wide query tiles (4)
NPAIR = HG // 2  # co-computed head pairs (4)

_CACHE = {}


def _split_excess_waits(nc):
    """Walrus in this container only accepts 1 sync-wait on CTRL-queue
    instructions (Drain etc.).  Hoist excess waits onto preceding nops on
    the same engine queue (program order makes this equivalent)."""
    n = 0
    for f in nc.m.functions:
        for bb in f.blocks:
            out = []
            for inst in bb.instructions:
                si = inst.sync_info
                limit = 1
                if si is not None and si.on_wait and len(si.on_wait) > limit:
                    waits = list(si.on_wait)
                    excess, keep = waits[:-limit], waits[-limit:]
                    for ci in range(0, len(excess), limit):
                        n += 1
                        out.append(mybir.InstNoOp(
                            name=f"waitsplit_{n}", opcode="nop", engine=inst.engine,
                            sync_info=mybir.SyncInfo(
                                on_wait=excess[ci:ci + limit], on_update=[]),
                        ))
                    inst.sync_info = mybir.SyncInfo(
                        on_wait=keep, on_update=list(si.on_update))
                out.append(inst)
            bb.instructions = out


def _r(ap):
    return ap


def _build(debug=False):
    nc = bass.Bass()
    x_in = nc.dram_tensor("x", [T, C], F32, kind="ExternalInput")
    wq_in = nc.dram_tensor("wq", [J, C], F32, kind="ExternalInput")
    wk_in = nc.dram_tensor("wk", [J, C], F32, kind="ExternalInput")
    wv_in = nc.dram_tensor("wv", [J, C], F32, kind="ExternalInput")
    wp_in = nc.dram_tensor("wp", [C, J], F32, kind="ExternalInput")
    bq_in = nc.dram_tensor("bq", [J], F32, kind="ExternalInput")
    bk_in = nc.dram_tensor("bk", [J], F32, kind="ExternalInput")
    bv_in = nc.dram_tensor("bv", [J], F32, kind="ExternalInput")
    y_out = nc.dram_tensor("y", [T, C], F32, kind="ExternalOutput")
    if debug:
        qt_d = nc.dram_tensor("qt_d", [P, JC, T], F32, kind="ExternalOutput")
        kt_d = nc.dram_tensor("kt_d", [P, JC, T], F32, kind="ExternalOutput")
        v_d = nc.dram_tensor("v_d", [P, NT, HG, D + 1], F32, kind="ExternalOutput")
        ot_d = nc.dram_tensor("ot_d", [P, JC, T], F32, kind="ExternalOutput")

    with TileContext(nc) as tc:
        with tc.tile_pool(name="persist", bufs=1) as persist:
            # persistent tensors (per-partition: 32+32+33+~3 KB)
            qt_t = persist.tile([P, JC, T], F32R, tag="qt")   # Q^T
            kt_t = persist.tile([P, JC, T], F32R, tag="kt")   # K^T
            v_t = persist.tile([P, NT, HG, D + 1], F32R, tag="v")  # V + ones col
            ident = persist.tile([P, P], F32, tag="ident")
            ones_row = persist.tile([1, P], F32R, tag="ones")
            bq_sb = persist.tile([P, JC], F32, tag="bq")
            bk_sb = persist.tile([P, JC], F32, tag="bk")
            bv_sb = persist.tile([1, J], F32, tag="bv")
            bv_r = persist.tile([1, J], F32R, tag="bvr")
            bv_bc = persist.tile([P, J], F32, tag="bvbc")

            # constants
            from concourse.masks import make_identity
            make_identity(nc, ident[:])
            ones_f32 = persist.tile([P, P], F32, tag="ones_f32")
            nc.gpsimd.memset(ones_f32[:], 1.0)
            nc.vector.tensor_copy(ones_row[:], ones_f32[0:1, :])
            nc.vector.tensor_copy(
                v_t[:, :, :, D:D + 1],
                ones_f32[:, None, None, 0:1].to_broadcast((P, NT, HG, 1)))
            nc.sync.dma_start(bq_sb[:], bq_in.rearrange("(o p) -> p o", p=P))
            nc.sync.dma_start(bk_sb[:], bk_in.rearrange("(o p) -> p o", p=P))
            nc.sync.dma_start(bv_sb[:], bv_in[None, :])

            # ---------------- phase 1: transposes + QKV projections ---------
            with (
                tc.tile_pool(name="nat", bufs=3) as nat_pool,
                tc.tile_pool(name="xt", bufs=2) as xt_pool,
                tc.tile_pool(name="wt", bufs=1) as wt_pool,
                tc.tile_pool(name="ps_tr", bufs=3, space="PSUM") as ps_tr,
                tc.tile_pool(name="ps_mm", bufs=3, space="PSUM") as ps_mm,
            ):
                # bv broadcast to all 128 partitions via K=1 matmul
                nc.vector.tensor_copy(bv_r[:], bv_sb[:])
                ps_bv = ps_mm.tile([P, J], F32, tag="mm")
                nc.tensor.matmul(ps_bv[:], lhsT=ones_row[:], rhs=bv_r[:],
                                 start=True, stop=True)
                nc.vector.tensor_copy(bv_bc[:], ps_bv[:])

                # weight transposes: w[j, c] -> w^T[c, j] chunks [P, CC, J]
                wts = {}
                for name, w_in in (("q", wq_in), ("k", wk_in), ("v", wv_in)):
                    wt = wt_pool.tile([P, CC, J], F32R, tag=f"w{name}t")
                    wts[name] = wt
                    for jt in range(JC):
                        wnat = nat_pool.tile([P, C], F32, tag="nat")
                        nc.sync.dma_start(wnat[:], w_in[jt * P:(jt + 1) * P, :])
                        for cc in range(CC):
                            pst = ps_tr.tile([P, P], F32, tag="tr")
                            nc.tensor.transpose(
                                pst[:], _r(wnat[:, cc * P:(cc + 1) * P]), _r(ident[:]))
                            nc.vector.tensor_copy(
                                wt[:, cc, jt * P:(jt + 1) * P], pst[:])

                # x^T per 256-column t block; project Q^T, K^T, V
                for tb in range(NTB):
                    xt = xt_pool.tile([P, CC, TBS], F32R, tag="xt")
                    for sub in range(TBS // P):
                        tt = tb * (TBS // P) + sub
                        xnat = nat_pool.tile([P, C], F32, tag="nat")
                        nc.sync.dma_start(xnat[:], x_in[tt * P:(tt + 1) * P, :])
                        for cc in range(CC):
                            pst = ps_tr.tile([P, P], F32, tag="tr")
                            nc.tensor.transpose(
                                pst[:], _r(xnat[:, cc * P:(cc + 1) * P]), _r(ident[:]))
                            nc.vector.tensor_copy(
                                xt[:, cc, sub * P:(sub + 1) * P], pst[:])

                    # Q^T[j, t] and K^T[j, t]
                    for name, dest, bias in (("q", qt_t, bq_sb), ("k", kt_t, bk_sb)):
                        wt = wts[name]
                        for jc in range(JC):
                            psq_full = ps_mm.tile([P, J], F32, tag="mm",
                                                  name=f"psq_{tb}_{name}_{jc}")
                            psq = psq_full[:, :TBS]
                            for cc in range(CC):
                                nc.tensor.matmul(
                                    psq[:],
                                    lhsT=_r(wt[:, cc, jc * P:(jc + 1) * P]),
                                    rhs=_r(xt[:, cc, :]),
                                    start=(cc == 0), stop=(cc == CC - 1))
                            nc.scalar.activation(
                                dest[:, jc, tb * TBS:(tb + 1) * TBS], psq[:],
                                AF.Identity, bias=bias[:, jc:jc + 1])

                    # V[t, j] (+ per-j bias broadcast over t)
                    for sub in range(TBS // P):
                        tt = tb * (TBS // P) + sub
                        psv = ps_mm.tile([P, J], F32, tag="mm")
                        for cc in range(CC):
                            nc.tensor.matmul(
                                psv[:],
                                lhsT=_r(xt[:, cc, sub * P:(sub + 1) * P]),
                                rhs=_r(wts["v"][:, cc, :]),
                                start=(cc == 0), stop=(cc == CC - 1))
                        nc.vector.tensor_tensor(
                            v_t[:, tt, :, 0:D],
                            psv.rearrange("p (h d) -> p h d", h=HG),
                            bv_bc.rearrange("p (h d) -> p h d", h=HG),
                            ALU.add)

            # ---------------- phases 2+3 -----------------------------------
            with tc.tile_pool(name="persist2", bufs=1) as persist2:
                ot_t = persist2.tile([P, JC, T], F32, tag="ot")  # O^T
                ot_r = persist2.tile([P, JC, T], F32R, tag="otr")

                with (
                    tc.tile_pool(name="e", bufs=3) as e_pool,
                    tc.tile_pool(name="qz", bufs=2) as qz_pool,
                    tc.tile_pool(name="zero", bufs=1) as zero_pool,
                    tc.tile_pool(name="tmp", bufs=3) as tmp_pool,
                    tc.tile_pool(name="rc", bufs=2) as rc_pool,
                    tc.tile_pool(name="ps_s", bufs=2, space="PSUM") as ps_s,
                    tc.tile_pool(name="ps_o", bufs=3, space="PSUM") as ps_o,
                    tc.tile_pool(name="ps_bc", bufs=1, space="PSUM") as ps_bc,
                ):
                    # HAM keeps the PE at half clock unless matmuls use the
                    # full 128-row array.  The d=64 contraction is padded to
                    # 128 by zeroing the other head-half of the Q operand
                    # (built once per (pair, q-tile), reused for every
                    # k-chunk); lhsT is then the full 2-head K^T chunk.
                    zt = zero_pool.tile([P, 512], F32, tag="zt")
                    nc.gpsimd.memset(zt[:], 0.0)
                    qz_init = {0: 0, 1: 0}
                    for pair in range(NPAIR):
                        for qt in range(NQ):
                            nk = (qt + 1) * 4
                            qs = slice(qt * 512, (qt + 1) * 512)
                            qzs = []
                            for half in range(2):
                                hs = slice(half * 64, (half + 1) * 64)
                                zs = slice((1 - half) * 64, (2 - half) * 64)
                                qz = qz_pool.tile(
                                    [P, 512], F32R, tag=f"qz{half}",
                                    name=f"qz{half}_{pair}_{qt}")
                                if qz_init[half] < 2:  # zero the dead half once per slot
                                    qz_init[half] += 1
                                    nc.vector.tensor_copy(
                                        qz[zs, :], zt[zs, :])
                                nc.vector.tensor_copy(
                                    qz[hs, :], qt_t[hs, pair, qs])
                                qzs.append(qz)
                            pso = [ps_o.tile([P, 512], F32, tag="o",
                                                 name=f"pso_{pair}_{qt}_{i}")
                                   for i in range(2)]
                            for kc in range(nk):
                                ks = slice(kc * P, (kc + 1) * P)
                                pss = ps_s.tile([P, 1024], F32, tag="s")
                                for half in range(2):
                                    nc.tensor.matmul(
                                        pss[:, half * 512:(half + 1) * 512],
                                        lhsT=_r(kt_t[:, pair, ks]),
                                        rhs=_r(qzs[half][:]),
                                        start=True, stop=True)
                                e = e_pool.tile([P, 1024], F32R, tag="e")
                                nc.scalar.activation(
                                    e[:], pss[:], AF.Exp, scale=0.125)
                                if kc >= qt * 4:  # diagonal: causal mask
                                    delta = (kc - qt * 4) * P
                                    nc.gpsimd.affine_select(
                                        out=e[:], in_=e[:],
                                        compare_op=ALU.is_ge, fill=0.0,
                                        base=-delta, channel_multiplier=-1,
                                        pattern=[[0, 2], [1, 512]])
                                for half in range(2):
                                    h = pair * 2 + half
                                    nc.tensor.matmul(
                                        pso[half][0:D + 1, :],
                                        lhsT=_r(v_t[:, kc, h, :]),
                                        rhs=_r(e[:, half * 512:(half + 1) * 512]),
                                        start=(kc == 0), stop=(kc == nk - 1))
                            # evict unnormalized O^T; softmax denominator
                            # handled as 1/d = exp(-ln d): ln of the PSUM ones
                            # row, K=1 matmul broadcast over 64 partitions,
                            # exp(-x) on eviction, one full-width multiply.
                            for half in range(2):
                                hs = slice(half * 64, (half + 1) * 64)
                                if half == 0:
                                    nc.vector.tensor_copy(
                                        ot_t[0:D, pair, qs], pso[half][0:D, :])
                                else:
                                    tmp = tmp_pool.tile([D, 512], F32, tag="tmp")
                                    nc.vector.tensor_copy(tmp[:], pso[half][0:D, :])
                                    nc.sync.dma_start(
                                        ot_t[D:2 * D, pair, qs], tmp[:])
                                lnden = rc_pool.tile(
                                    [1, 512], F32R, tag="ln",
                                    name=f"ln_{pair}_{qt}_{half}")
                                nc.scalar.activation(
                                    lnden[:], pso[half][D:D + 1, :], AF.Ln)
                                psb = ps_bc.tile([P, 512], F32, tag="bc",
                                                 name=f"psb_{pair}_{qt}_{half}")
                                nc.tensor.matmul(
                                    psb[:, :], lhsT=ones_row[0:1, :],
                                    rhs=lnden[:], start=True, stop=True)
                                bcx = tmp_pool.tile([P, 512], F32, tag="bcx",
                                                    name=f"bcx_{pair}_{qt}_{half}")
                                nc.scalar.activation(
                                    bcx[hs, :], psb[hs, :], AF.Exp, scale=-1.0)
                                nc.vector.tensor_tensor(
                                    ot_r[hs, pair, qs], ot_t[hs, pair, qs],
                                    bcx[hs, :], ALU.mult)

                if debug:
                    nc.sync.dma_start(qt_d[:], qt_t[:].bitcast(F32))
                    nc.sync.dma_start(kt_d[:], kt_t[:].bitcast(F32))
                    nc.sync.dma_start(v_d[:], v_t[:].bitcast(F32))
                    nc.sync.dma_start(ot_d[:], ot_r[:].bitcast(F32))
                # ------------ phase 3: output projection --------------------
                with (
                    tc.tile_pool(name="nat3", bufs=2) as nat3,
                    tc.tile_pool(name="wpt", bufs=1) as wpt_pool,
                    tc.tile_pool(name="yout", bufs=2) as y_pool,
                    tc.tile_pool(name="ps_tr3", bufs=2, space="PSUM") as ps_tr3,
                    tc.tile_pool(name="ps_y", bufs=4, space="PSUM") as ps_y,
                ):
                    wpt = wpt_pool.tile([P, JC, C], F32R, tag="wpt")
                    for ct in range(C // P):
                        wnat = nat3.tile([P, J], F32, tag="nat3")
                        nc.sync.dma_start(wnat[:], wp_in[ct * P:(ct + 1) * P, :])
                        for jc in range(JC):
                            pst = ps_tr3.tile([P, P], F32, tag="tr3")
                            nc.tensor.transpose(
                                pst[:], _r(wnat[:, jc * P:(jc + 1) * P]), _r(ident[:]))
                            nc.vector.tensor_copy(
                                wpt[:, jc, ct * P:(ct + 1) * P], pst[:])

                    for tt in range(NT):
                        ts = slice(tt * P, (tt + 1) * P)
                        ytile = y_pool.tile([P, C], F32, tag="y")
                        for nh in range(2):
                            psy = ps_y.tile([P, 512], F32, tag="ps_y")
                            for jc in range(JC):
                                nc.tensor.matmul(
                                    psy[:],
                                    lhsT=ot_r[:, jc, ts],
                                    rhs=_r(wpt[:, jc, nh * 512:(nh + 1) * 512]),
                                    start=(jc == 0), stop=(jc == JC - 1))
                            nc.vector.tensor_copy(
                                ytile[:, nh * 512:(nh + 1) * 512], psy[:])
                        nc.sync.dma_start(y_out[ts, :], ytile[:])

    _split_excess_waits(nc)
    return nc


def _get_nc():
    if "nc" not in _CACHE:
        _CACHE["nc"] = _build()
    return _CACHE["nc"]


def kernel(x, Wq, bq, Wk, bk, Wv, bv, Wp, bp, **_unused):
    x = np.ascontiguousarray(np.asarray(x, dtype=np.float32))
    Wq = np.asarray(Wq, dtype=np.float32)
    Wk = np.asarray(Wk, dtype=np.float32)
    Wv = np.asarray(Wv, dtype=np.float32)
    Wp = np.asarray(Wp, dtype=np.float32)
    bq = np.asarray(bq, dtype=np.float32)
    bk = np.asarray(bk, dtype=np.float32)
    bv = np.asarray(bv, dtype=np.float32)
    bp = np.asarray(bp, dtype=np.float32)

    nc = _get_nc()
    in_maps = []
    for c in range(8):
        b, g = c // 2, c % 2
        js = slice(g * J, (g + 1) * J)
        in_maps.append({
            "x": np.ascontiguousarray(x[b]),
            "wq": np.ascontiguousarray(Wq[js, :]),
            "wk": np.ascontiguousarray(Wk[js, :]),
            "wv": np.ascontiguousarray(Wv[js, :]),
            "wp": np.ascontiguousarray(Wp[:, js]),
            "bq": np.ascontiguousarray(bq[js]),
            "bk": np.ascontiguousarray(bk[js]),
            "bv": np.ascontiguousarray(bv[js]),
        })
    res = run_bass_kernel_spmd(nc, in_maps, list(range(8)))
    out = np.empty((4, T, C), dtype=np.float32)
    for b in range(4):
        out[b] = res.results[2 * b]["y"] + res.results[2 * b + 1]["y"] + bp
    return out
